# revision 50
# baseline (speedup 1.0000x reference)
"""Multi-head attention layer on 8 Trainium2 NeuronCores.

Sharding (zero-communication): core c -> (batch c//2, head-group c%2), i.e.
each core owns 8 of the 16 heads (512 of 1024 hidden dims) for one batch
element.  Per core: QKV projections for its heads, full softmax attention,
and a partial output projection (row-parallel over Wo).  The host sums the
two partial outputs per batch and adds the constant bias terms
(bo + bv @ Wo.T -- the value bias commutes through softmax since attention
rows sum to 1), so no on-device collectives are needed.

dtypes: all matmul operands fp16; PSUM accumulation + softmax in fp32.

Structure: 16 blocks = (head-pair c, l-block of 512), lc0 blocks first.
Per block, 16 s-tiles (st): one score MM per half -- the two halves
contract over only E=64 so they run CONCURRENTLY on the 64-row-tiled PE
array (row groups auto-derived from base partitions).  exp splits across
engines: h0's tile on ACT (true exp), h64's on DVE via a Schraudolph fp16
bit-trick (bits = rint(a*logit+b) as int16 reinterpreted fp16, ~3% per
weight, ~6.6e-3 end-to-end).  PSUM (the whole point of the 512 blocks):
scores 4x[128,512] slots (double-buffered per half -> exp latency is OFF
the critical chain), AV 2x[65,512], projections 2x[128,512] dedicated
slots (never contending with score slots).  Softmax normalization per
block: the two halves' sums (fp16) are broadcast to [128,512] by two
serialized outer-product matmuls into a proj slot (no DMA anywhere on
this path), reciprocal runs on all 128 DVE lanes off PSUM, and the attT
multiply goes to the otherwise idle GPSIMD engine -- all deferred into
the next block's weave with ordering edges behind its exps, so the
normalization can never head-of-line-block the DVE FIFO whose exps
recycle the PSUM score slots.  A dep-free warm-up matmul stream holds
the PE HAM un-throttled through the initial DMA-bound ramp.  Output is
fp16 (halves the out-DMA); the host sums the two partial outputs per
batch in fp32.
"""

import os
import numpy as np

B, L, S = 4, 2048, 2048
D, NH, E = 1024, 16, 64
N_CORES = 8
HG = 2
LH = NH // HG         # 8 local heads
DH = LH * E           # 512
LC = 1024
SCALE = 1.0 / np.sqrt(E)

# Schraudolph fp16 exp: bits = rint(A_SCH * raw_score + B_SCH) viewed as
# fp16 ~= exp(SCALE * raw_score).  A = 1024/ln(2) * SCALE; B = 1024*15 +
# sigma with sigma=-44.75 balancing the mantissa-interp error to ~+-3%.
A_SCH = float(1024.0 / np.log(2.0) * SCALE)
B_SCH = float(1024.0 * 15 - 44.75)

_compiled = {}
last_exec_time_ns = None
last_results = None


def _build():
    import concourse.bass as bass
    import concourse.mybir as mybir
    import concourse.tile as tile
    from concourse import bacc
    from concourse.dve_ops import RECIPROCAL_APPROX_NR

    f32 = mybir.dt.float32
    fp16 = mybir.dt.float16

    nc = bacc.Bacc("TRN2", target_bir_lowering=False, debug=False,
                   num_devices=N_CORES)

    xqT = nc.dram_tensor("xqT", [D, L], fp16, kind="ExternalInput").ap()
    xkT = nc.dram_tensor("xkT", [D, S], fp16, kind="ExternalInput").ap()
    xvT = nc.dram_tensor("xvT", [D, S], fp16, kind="ExternalInput").ap()
    wqT = nc.dram_tensor("wqT", [D, DH], fp16, kind="ExternalInput").ap()
    wkT = nc.dram_tensor("wkT", [D, DH], fp16, kind="ExternalInput").ap()
    wvT = nc.dram_tensor("wvT", [D, DH], fp16, kind="ExternalInput").ap()
    woT = nc.dram_tensor("woT", [DH, D], fp16, kind="ExternalInput").ap()
    bq_d = nc.dram_tensor("bq", [DH], f32, kind="ExternalInput").ap()
    bk_d = nc.dram_tensor("bk", [DH], f32, kind="ExternalInput").ap()
    out_d = nc.dram_tensor("out", [L, D], fp16,
                           kind="ExternalOutput").ap()

    Exp = mybir.ActivationFunctionType.Exp
    Ident = mybir.ActivationFunctionType.Identity
    Copy = mybir.ActivationFunctionType.Copy

    with tile.TileContext(nc) as tc:
        with (
            tc.tile_pool(name="res", bufs=1) as res,
            tc.tile_pool(name="xsq", bufs=16) as xsq,
            tc.tile_pool(name="xsv", bufs=16) as xsv,
            tc.tile_pool(name="pp", bufs=6) as pp,
            tc.tile_pool(name="os", bufs=4) as osp,
            tc.tile_pool(name="sm", bufs=1) as sm,
            tc.tile_pool(name="sm2", bufs=1) as sm2,
            tc.tile_pool(name="avs", bufs=2) as avs,
            tc.tile_pool(name="psS", bufs=4, space="PSUM") as psS,
            tc.tile_pool(name="psP", bufs=2, space="PSUM") as psP,
            tc.tile_pool(name="psAV", bufs=2, space="PSUM") as psAV,
        ):
            # ---- resident weights / biases ----
            bq_sb = res.tile([128, DH // 128], f32, tag="bq")
            bk_sb = res.tile([128, DH // 128], f32, tag="bk")
            nc.sync.dma_start(bq_sb[:], bq_d.rearrange("(c p) -> p c", p=128))
            nc.sync.dma_start(bk_sb[:], bk_d.rearrange("(c p) -> p c", p=128))
            wk_r = res.tile([128, D // 128, DH], fp16, tag="wkr")
            wq_r = res.tile([128, D // 128, DH], fp16, tag="wqr")
            wkT_r = wkT.rearrange("(c p) n -> p c n", p=128)
            wqT_r = wqT.rearrange("(c p) n -> p c n", p=128)
            nc.sync.dma_start(wk_r[:, :, 0:128], wkT_r[:, :, 0:128])
            xk_sb = res.tile([128, D // 128, S], fp16, tag="xk")
            for d in range(8):
                nc.sync.dma_start(xk_sb[:, d, 0:512],
                                  xkT[d * 128:(d + 1) * 128, 0:512])
            nc.sync.dma_start(wq_r[:, :, 0:128], wqT_r[:, :, 0:128])
            wv_sb = res.tile([128, D // 128, DH], fp16, tag="wv")
            wo_sb = res.tile([128, DH // 128, D], fp16, tag="wo")
            ones_f = res.tile([128, 128], f32, tag="onesf")
            nc.vector.memset(ones_f[:], 1.0)
            ones16 = res.tile([1, 128], fp16, tag="ones16")
            nc.vector.memset(ones16[:], 1.0)

            def load_late_residents():
                # xk blocks first: k_chunk(0,1..3) read them within ~5-15us,
                # while the wk/wq remainders are first read ~20us in and wo
                # only in the lc1 phase
                for d in range(8):
                    nc.sync.dma_start(
                        xk_sb[:, d, 512:1536],
                        xkT[d * 128:(d + 1) * 128, 512:1536])
                nc.sync.dma_start(wk_r[:, :, 128:256], wkT_r[:, :, 128:256])
                for d in range(8):
                    nc.sync.dma_start(xk_sb[:, d, 1536:2048],
                                      xkT[d * 128:(d + 1) * 128, 1536:2048])
                nc.sync.dma_start(wq_r[:, :, 128:256], wqT_r[:, :, 128:256])
                nc.sync.dma_start(wk_r[:, :, 256:512], wkT_r[:, :, 256:512])
                nc.sync.dma_start(wq_r[:, :, 256:512], wqT_r[:, :, 256:512])
                nc.sync.dma_start(
                    wo_sb[:], woT.rearrange("(c p) n -> p c n", p=128))

            qT_sb = res.tile([128, DH // 128, L], fp16, tag="qT")
            kT_sb = res.tile([128, DH // 128, S], fp16, tag="kT")
            v1_sb = res.tile([128, S // 128, LH, E + 1], fp16, tag="v1")
            nc.vector.tensor_copy(
                v1_sb[:, :, :, E:E + 1],
                ones_f[:, 0:S // 128 * LH].rearrange(
                    "p (s h o) -> p s h o", h=LH, o=1))

            attT = {}
            attT[0] = res.tile([128, DH // 128, LC], fp16, tag="attT0",
                               name="attT0")
            attT[1] = res.tile([128, DH // 128, LC], fp16, tag="attT1",
                               name="attT1")

            # ---- streamed x tiles ----
            xq_t = {}

            def load_xq(bl):
                for d in range(8):
                    t = xsq.tile([128, 512], fp16, tag="xq",
                                 name=f"xq{bl}_{d}")
                    nc.sync.dma_start(
                        t[:], xqT[d * 128:(d + 1) * 128,
                                  bl * 512:(bl + 1) * 512])
                    xq_t[(bl, d)] = t

            xv_t = {}

            def load_xv(bl):
                for d in range(8):
                    t = xsv.tile([128, 512], fp16, tag="xv",
                                 name=f"xv{bl}_{d}")
                    nc.sync.dma_start(
                        t[:], xvT[d * 128:(d + 1) * 128,
                                  bl * 512:(bl + 1) * 512])
                    xv_t[(bl, d)] = t

            # ---- projection chunks: dedicated psP slots, [128,512] ----
            def k_chunk(dh, bl):
                prj = psP.tile([128, 512], f32, tag="pr", name="kprj")
                for d in range(8):
                    nc.tensor.matmul(
                        prj[:],
                        wk_r[:, d, dh * 128:(dh + 1) * 128],
                        xk_sb[:, d, bl * 512:(bl + 1) * 512],
                        start=(d == 0), stop=(d == 7))
                nc.scalar.activation(
                    kT_sb[:, dh, bl * 512:(bl + 1) * 512], prj[:],
                    Ident, bias=bk_sb[:, dh:dh + 1])

            def q_chunk(dh, bl):
                prj = psP.tile([128, 512], f32, tag="pr", name="qprj")
                for d in range(8):
                    nc.tensor.matmul(
                        prj[:],
                        wq_r[:, d, dh * 128:(dh + 1) * 128],
                        xq_t[(bl, d)][:], start=(d == 0), stop=(d == 7))
                nc.scalar.activation(
                    qT_sb[:, dh, bl * 512:(bl + 1) * 512], prj[:],
                    Ident, bias=bq_sb[:, dh:dh + 1])

            def v_chunk(st):
                bl, st4 = st // 4, st % 4
                vp = psP.tile([128, 512], f32, tag="pr", name="vprj")
                for d in range(8):
                    nc.tensor.matmul(
                        vp[:],
                        xv_t[(bl, d)][:, st4 * 128:(st4 + 1) * 128],
                        wv_sb[:, d, :], start=(d == 0), stop=(d == 7))
                nc.vector.tensor_copy(
                    v1_sb[:, st, :, 0:E],
                    vp[:].rearrange("p (h e) -> p h e", h=LH))

            def out_chunk(lc, ls, n2, dve_evict=False):
                op = psP.tile([128, 512], f32, tag="pr", name="op")
                for dhc in range(DH // 128):
                    nc.tensor.matmul(
                        op[:],
                        attT[lc][:, dhc, ls * 128:(ls + 1) * 128],
                        wo_sb[:, dhc, n2 * 512:(n2 + 1) * 512],
                        start=(dhc == 0), stop=(dhc == DH // 128 - 1))
                row = lc * LC + ls * 128
                o_sb = osp.tile([128, 512], fp16, tag="o")
                if dve_evict:
                    nc.vector.tensor_copy(o_sb[:], op[:])
                else:
                    nc.scalar.activation(o_sb[:], op[:], Copy)
                nc.sync.dma_start(
                    out_d[row:row + 128, n2 * 512:(n2 + 1) * 512], o_sb[:])

            # ---- per-block softmax normalization ----
            pending_fin = []

            def _drain_block(c, lb, av, final_block=False):
                lc, off = lb // 2, (lb % 2) * 512
                sums = sm.tile([1, 2, 512], fp16, tag="sums0", name="sums")
                av_sb = avs.tile([128, 512], f32, tag="avs", name="av_sb")
                # av copies first: they release the PSUM av slots the next
                # block's first AV accumulation is waiting on
                nc.scalar.activation(av_sb[0:E, :], av[0][0:E, :], Copy)
                nc.vector.tensor_copy(av_sb[E:2 * E, :], av[1][0:E, :])
                nc.scalar.activation(sums[0:1, 0, :], av[0][E:E + 1, :],
                                     Copy)
                nc.vector.tensor_copy(sums[0:1, 1, :], av[1][E:E + 1, :])
                rb_sb = sm2.tile([128, 512], f32, tag="rb", name="rb_sb")
                scr = sm.tile([128, 512], f32, tag="scr", name="scr")

                # no DMA anywhere: broadcast the (fp16) sums to [128,512]
                # via two tiny outer-product matmuls into a proj slot, then
                # reciprocal on all 128 DVE lanes off PSUM.
                def fin_bc():
                    rbp = psP.tile([128, 512], f32, tag="pr", name="rbp")
                    prev_mm = None
                    for half in (0, 1):
                        mm = nc.tensor.matmul(
                            rbp[half * 64:(half + 1) * 64, :],
                            ones16[0:1, 0:64],
                            sums[0:1, half, :],
                            start=True, stop=True)
                        if prev_mm is not None:
                            # the two col-tiled halves target the SAME
                            # PSUM bank; concurrent drains there are a
                            # fatal HW collision -- force serialization
                            tile.add_dep_helper(mm.ins, prev_mm.ins,
                                                sync=True,
                                                reason="rbp same-bank")
                        prev_mm = mm
                    return rbp

                def fin_fast(rbp, after):
                    i = nc.vector.reciprocal_approx_fast(
                        out=scr[:], in_=rbp[:])
                    tile.add_dep_helper(i.ins, after.ins, sync=False,
                                        reason="recip after exps")

                def fin_nr(rbp, after):
                    i = nc.vector._custom_dve(
                        RECIPROCAL_APPROX_NR, out=rb_sb[:], in0=rbp[:],
                        in1=scr[:], s0=2.0)
                    tile.add_dep_helper(i.ins, after.ins, sync=False,
                                        reason="recip-nr after exps")

                def fin_mul(after):
                    eng = nc.vector if final_block else nc.gpsimd
                    eng.tensor_mul(attT[lc][:, c, off:off + 512],
                                   av_sb[:], rb_sb[:])

                if final_block:
                    rbp = fin_bc()
                    nc.vector.reciprocal_approx_fast(out=scr[:], in_=rbp[:])
                    nc.vector._custom_dve(
                        RECIPROCAL_APPROX_NR, out=rb_sb[:], in0=rbp[:],
                        in1=scr[:], s0=2.0)
                    fin_mul(None)
                else:
                    pending_fin.append((fin_bc, fin_fast, fin_nr, fin_mul))

            # ---- one attention block (2 heads, one 512-wide l block) ----
            def attention_block(c, lb, weave_dma, weave, weave_av=None,
                                final_block=False):
                h0, h1 = 2 * c, 2 * c + 1
                av = [psAV.tile([E + 1, 512], f32, tag="av", name=f"av{i}")
                      for i in (0, 1)]

                def emit_av(st, first, last):
                    for half, h in ((0, h0), (1, h1)):
                        nc.tensor.matmul(
                            av[half][:], v1_sb[:, st, h, :],
                            P_of[st][half][:], start=first, stop=last)
                    if last:
                        _drain_block(c, lb, av, final_block=final_block)

                P_of = {}
                for st in range(16):
                    for w in weave_dma[st]:
                        w()
                    sc = [psS.tile([128, 512], f32, tag="sc",
                                   name=f"sc{i}") for i in (0, 1)]
                    # the two halves' score MMs are emitted back-to-back:
                    # disjoint 64-row groups + disjoint PSUM banks ->
                    # concurrent execution on the row-tiled PE array
                    for half, p0 in ((0, 0), (1, 64)):
                        nc.tensor.matmul(
                            sc[half][:],
                            kT_sb[p0:p0 + 64, c, st * 128:(st + 1) * 128],
                            qT_sb[p0:p0 + 64, c, lb * 512:(lb + 1) * 512],
                            start=True, stop=True)
                    P_of[st] = [pp.tile([128, 512], fp16, tag="P",
                                        name=f"P{i}") for i in (0, 1)]
                    nc.scalar.activation(P_of[st][0][:], sc[0][:],
                                         Exp, scale=SCALE)
                    e1 = nc.vector.tensor_scalar(
                        P_of[st][1][:].bitcast(mybir.dt.int16), sc[1][:],
                        A_SCH, B_SCH,
                        mybir.AluOpType.mult, mybir.AluOpType.add)
                    if pending_fin:
                        if st == 3:
                            pending_fin[0] = (pending_fin[0][0](),) + \
                                pending_fin[0][1:]
                        elif st == 4:
                            pending_fin[0][1](pending_fin[0][0], e1)
                        elif st == 5:
                            pending_fin[0][2](pending_fin[0][0], e1)
                        elif st == 6:
                            pending_fin.pop(0)[3](e1)
                    for w in weave[st]:
                        w()
                    if weave_av:
                        for w in weave_av[st]:
                            w()
                    if st > 1:
                        emit_av(st - 2, first=(st == 2), last=False)
                emit_av(14, first=False, last=False)
                emit_av(15, first=False, last=True)

            # ---- emission ----
            load_xq(0)
            load_xq(1)
            # warm-up stream: dep-free matmuls that keep the PE HAM
            # un-throttled through the initial DMA wait, so the first real
            # matmuls run at 2.4 GHz instead of 1.2
            warm = psP.tile([64, 128], f32, tag="pr", name="warm")
            for _ in range(48):
                nc.tensor.matmul(warm[:], ones_f[0:1, 0:64],
                                 ones_f[0:1, 0:128], start=True, stop=True)
            k_chunk(0, 0)
            q_chunk(0, 0)
            nc.sync.dma_start(wv_sb[:],
                              wvT.rearrange("(c p) n -> p c n", p=128))
            load_xv(0)
            load_late_residents()

            # blocks: (c, lb) with lb the absolute 512-wide l block; lc0
            # blocks first so attT[0] completes before the lc1 phase where
            # its out-proj chunks weave in.
            BLOCKS = [(c, sub) for c in range(4) for sub in (0, 1)] + \
                     [(c, 2 + sub) for c in range(4) for sub in (0, 1)]

            def weave_for(idx):
                wd = [[] for _ in range(16)]
                w = [[] for _ in range(16)]
                wav = [[] for _ in range(16)]
                c, lb = BLOCKS[idx]
                if idx == 0:
                    for st in range(16):
                        wav[st].append(lambda st=st: v_chunk(st))
                    wd[1].append(lambda: load_xv(1))
                    wd[5].append(lambda: load_xv(2))
                    wd[9].append(lambda: load_xv(3))
                    w[2].append(lambda: k_chunk(0, 1))
                    w[6].append(lambda: k_chunk(0, 2))
                    w[10].append(lambda: k_chunk(0, 3))
                    w[13].append(lambda: q_chunk(0, 1))
                elif idx in (1, 3, 5):
                    cn = (idx + 1) // 2  # next head pair
                    w[2].append(lambda cn=cn: k_chunk(cn, 0))
                    w[6].append(lambda cn=cn: k_chunk(cn, 1))
                    w[10].append(lambda cn=cn: k_chunk(cn, 2))
                    w[14].append(lambda cn=cn: k_chunk(cn, 3))
                    w[12].append(lambda cn=cn: q_chunk(cn, 0))
                elif idx in (2, 4, 6):
                    w[8].append(lambda c=c: q_chunk(c, 1))
                elif idx == 7:
                    wd[1].append(lambda: load_xq(2))
                    wd[5].append(lambda: load_xq(3))
                    w[8].append(lambda: q_chunk(0, 2))
                    w[12].append(lambda: q_chunk(0, 3))
                else:
                    j = idx - 8
                    # q chunk for the block two ahead (same cadence)
                    if j + 2 < 8:
                        cn, sub = (j + 2) // 2, (j + 2) % 2
                        w[2].append(lambda cn=cn, sub=sub:
                                    q_chunk(cn, 2 + sub))
                    # two lc0 out-proj chunks per lc1 block
                    combo0 = 2 * j
                    for i, combo in enumerate((combo0, combo0 + 1)):
                        ls, n2 = combo // 2, combo % 2
                        w[8 + 3 * i].append(
                            lambda ls=ls, n2=n2: out_chunk(0, ls, n2))
                return wd, w, wav

            for idx in range(16):
                c, lb = BLOCKS[idx]
                wd, w, wav = weave_for(idx)
                attention_block(c, lb, wd, w,
                                weave_av=wav if idx == 0 else None,
                                final_block=(idx == 15))

            # tail: lc1 out-projection, evictions alternate ACT/DVE so
            # the 2-slot psP pipeline never waits on one engine
            for ls in range(LC // 128):
                for n2 in range(2):
                    out_chunk(1, ls, n2, dve_evict=(n2 == 1))

    nc.compile()
    return nc


def _get_nc():
    if "nc" not in _compiled:
        _compiled["nc"] = _build()
    return _compiled["nc"]


def kernel(queries, keys, values, Wq, bq, Wk, bk, Wv, bv, Wo, bo):
    global last_exec_time_ns, last_results
    from concourse import bass_utils

    queries = np.asarray(queries, dtype=np.float32)
    keys = np.asarray(keys, dtype=np.float32)
    values = np.asarray(values, dtype=np.float32)
    Wq, bq = np.asarray(Wq, np.float32), np.asarray(bq, np.float32)
    Wk, bk = np.asarray(Wk, np.float32), np.asarray(bk, np.float32)
    Wv, bv = np.asarray(Wv, np.float32), np.asarray(bv, np.float32)
    Wo, bo = np.asarray(Wo, np.float32), np.asarray(bo, np.float32)

    nc = _get_nc()

    in_maps = []
    for c in range(N_CORES):
        b, g = c // HG, c % HG
        sl = slice(g * DH, (g + 1) * DH)
        in_maps.append({
            "xqT": np.ascontiguousarray(queries[b].T).astype(np.float16),
            "xkT": np.ascontiguousarray(keys[b].T).astype(np.float16),
            "xvT": np.ascontiguousarray(values[b].T).astype(np.float16),
            "wqT": np.ascontiguousarray(Wq[sl, :].T).astype(np.float16),
            "wkT": np.ascontiguousarray(Wk[sl, :].T).astype(np.float16),
            "wvT": np.ascontiguousarray(Wv[sl, :].T).astype(np.float16),
            "woT": np.ascontiguousarray(Wo[:, sl].T).astype(np.float16),
            "bq": np.ascontiguousarray(bq[sl]),
            "bk": np.ascontiguousarray(bk[sl]),
        })

    trace = bool(os.environ.get("KERNEL_TRACE"))
    if trace:
        try:
            import antenv.axon_hooks  # noqa: F401
        except ImportError:
            trace = False
    res = bass_utils.run_bass_kernel_spmd(
        nc, in_maps, core_ids=list(range(N_CORES)), trace=trace)
    last_exec_time_ns = res.exec_time_ns
    last_results = res

    const = (bo + bv @ Wo.T).astype(np.float32)
    out = np.empty((B, L, D), np.float32)
    for b in range(B):
        out[b] = (res.results[HG * b]["out"].astype(np.float32)
                  + res.results[HG * b + 1]["out"].astype(np.float32)
                  + const)
    return out
